# revision 27
# baseline (speedup 1.0000x reference)
"""GCNRouting2Hop on 8 trn2 NeuronCores (Bass/Tile SPMD kernel).

Sharding: dst-node partition (2500 nodes/core, 20 blocks of 128).
Self-loops are folded into the edge list (norm = dinv^2) on host.

Layer 1 has NO dynamic gathers: gather indices are host-known, so x is
pre-expanded into a per-edge stream xg [128, CH, 128] bf16 and streamed
on the SP HWDGE queue while one-hot scatter tiles stream on the ACT
HWDGE queue (both issued P blocks ahead so stream issues are not stuck
behind LN work in the in-order engine queues). Per chunk the
TensorEngine accumulates zT += xg.T @ onehot in PSUM.

h is cast to fp8e4 and AllGathered in 5 slices (one Shared-output
collective per slice tensor), triggered as L1 blocks complete. Edges
are sorted per block by the AG slice of their source, so layer-2
gathers are emitted slice-major on the gpsimd queue right behind the
AG triggers: slice-s gathers (one per block, fp8 256B descriptors)
drain into a rotating slice staging tile while later AG slices are
still in flight. Layer-2 scatter matmuls run slice-major too
(lhsT=gh_half fp8 x rhs=onehot bf16 -> [feat,dst] PSUM per
(block,slice) session), accumulating into per-block SBUF zacc tiles,
so the PE consumes data in drain order. A final block-major pass does
the slice-4 session + W2 dense + identity-residual + LayerNorm.
"""
import os
import sys
import types

sys.path.insert(0, '/opt/trn_rl_repo')
import numpy as np


def _install_axon_hooks_shim():
    try:
        import antenv
    except ImportError:
        return
    if hasattr(antenv, 'axon_hooks') or 'antenv.axon_hooks' in sys.modules:
        return
    try:
        from trn_agent_boot.trn_boot import _ntff_profile_via_ctypes
        hook = _ntff_profile_via_ctypes('/opt/axon/libaxon_pjrt.so')
    except Exception:
        hook = None
    mod = types.ModuleType('antenv.axon_hooks')
    mod._hook = hook
    mod.get_axon_ntff_profile_hook = lambda: mod._hook

    def set_axon_ntff_profile_hook(h):
        mod._hook = h

    mod.set_axon_ntff_profile_hook = set_axon_ntff_profile_hook
    sys.modules['antenv.axon_hooks'] = mod
    antenv.axon_hooks = mod


_install_axon_hooks_shim()

import ml_dtypes
from concourse import bacc, mybir, tile
from concourse.masks import make_identity
from concourse.bass_utils import run_bass_kernel_spmd

N = 20000
NC = 8
NPC = N // NC              # 2500 dst nodes per core
NBLK = (NPC + 127) // 128  # 20 blocks of 128 dst nodes
DIN = 128
DH = 256
LN_EPS = 1e-5

# AllGather slice layout: 5 slices of local rows per core
SLQ = (512, 512, 512, 512, NPC - 2048)   # 512,512,512,512,452
SLO = (0, 512, 1024, 1536, 2048)
SL_LAST_BLOCK = (3, 7, 11, 15, 19)       # trigger slice q after this block

LAST_EXEC_TIME_NS = None
_prog_cache = {}

f32 = mybir.dt.float32
bf16 = mybir.dt.bfloat16
fp8 = mybir.dt.float8e4
i16 = mybir.dt.int16

# cstf fp32 [128, 4, 256] rows; cstb bf16 [128, 7, 256]
(F_G1, F_BE1, F_G2, F_BE2) = range(4)
(B_W1, B_W2A, B_W2B, B_WRES, B_B1, B_BRES, B_B2) = range(7)


def _ln(nc, epi, u, gt, bt, out_tile, eps_ap):
    """LayerNorm over free axis; nodes on partitions. DVE kept off the
    slow TensorScalarPtr path: reductions on ACT accumulators, the
    normalize on ACT Identity with per-partition scale/bias."""
    sq = epi.tile([128, DH], f32, tag="sq")
    s1 = epi.tile([128, 1], f32, tag="s1")
    s2 = epi.tile([128, 1], f32, tag="s2")
    nc.vector.tensor_reduce(out=s1[:], in_=u[:], axis=mybir.AxisListType.X,
                            op=mybir.AluOpType.add)
    nc.scalar.activation(sq[:], u[:], mybir.ActivationFunctionType.Square,
                         accum_out=s2[:])
    mu = epi.tile([128, 1], f32, tag="mu")
    nc.vector.tensor_scalar(out=mu[:], in0=s1[:], scalar1=1.0 / DH,
                            scalar2=None, op0=mybir.AluOpType.mult)
    var = epi.tile([128, 1], f32, tag="var")
    musq = epi.tile([128, 1], f32, tag="musq")
    nc.vector.tensor_tensor(out=musq[:], in0=mu[:], in1=mu[:],
                            op=mybir.AluOpType.mult)
    nc.vector.tensor_scalar(out=var[:], in0=s2[:], scalar1=1.0 / DH,
                            scalar2=None, op0=mybir.AluOpType.mult)
    nc.vector.tensor_tensor(out=var[:], in0=var[:], in1=musq[:],
                            op=mybir.AluOpType.subtract)
    std = epi.tile([128, 1], f32, tag="std")
    nc.scalar.activation(std[:], var[:], mybir.ActivationFunctionType.Sqrt,
                         bias=eps_ap)
    rstd = epi.tile([128, 1], f32, tag="rstd")
    nc.vector.reciprocal(rstd[:], std[:])
    nmr = epi.tile([128, 1], f32, tag="nmr")
    nc.vector.tensor_tensor(out=nmr[:], in0=mu[:], in1=rstd[:],
                            op=mybir.AluOpType.mult)
    nc.vector.tensor_scalar(out=nmr[:], in0=nmr[:], scalar1=-1.0,
                            scalar2=None, op0=mybir.AluOpType.mult)
    un = epi.tile([128, DH], f32, tag="un")
    nc.scalar.activation(un[:], u[:], mybir.ActivationFunctionType.Identity,
                         bias=nmr[:], scale=rstd[:])
    g = epi.tile([128, DH], f32, tag="g")
    nc.vector.tensor_tensor(out=g[:], in0=un[:], in1=gt,
                            op=mybir.AluOpType.mult)
    nc.vector.tensor_tensor(out=out_tile[:], in0=g[:], in1=bt,
                            op=mybir.AluOpType.add)


def _build_program(pad5, hgdt_name, shared):
    hgdt = {"fp8": fp8, "bf16": bf16}[hgdt_name]
    pad5 = np.asarray(pad5, np.int64)
    R = tuple(int(v) for v in pad5.sum(axis=1))
    offs = np.concatenate([[0], np.cumsum(R)]).astype(np.int64)
    E_pad = int(offs[-1])
    CH = E_pad // 128
    smax = int(max(R)) // 128
    soff5 = np.zeros((NBLK, 5), np.int64)
    soff5[:, 1:] = np.cumsum(pad5, axis=1)[:, :-1]
    # chunks per slice staging tile and per-(slice, block) chunk offsets
    SCH = [int(pad5[:, s].sum()) // 128 for s in range(5)]
    SCHmax = max(SCH)
    boff = np.zeros((NBLK, 5), np.int64)
    boff[1:, :] = np.cumsum(pad5 // 128, axis=0)[:-1, :]

    nc = bacc.Bacc("TRN2", target_bir_lowering=False, debug=False,
                   num_devices=NC, num_swdge_queues=4)
    xg_in = nc.dram_tensor("xg", [128, CH, DIN], bf16, kind="ExternalInput")
    oh_in = nc.dram_tensor("ohb", [128, CH, 128], bf16, kind="ExternalInput")
    idx2_in = nc.dram_tensor("idx2", [128, E_pad // 16], i16,
                             kind="ExternalInput")
    xT_in = nc.dram_tensor("xT", [128, NBLK * 128], bf16,
                           kind="ExternalInput")
    cstf_in = nc.dram_tensor("cstf", [128, 4, DH], f32, kind="ExternalInput")
    cstb_in = nc.dram_tensor("cstb", [128, 7, DH], bf16, kind="ExternalInput")
    cnt_in = nc.dram_tensor("cnt5", [1, NBLK * 5], mybir.dt.int32,
                            kind="ExternalInput")
    out_t = nc.dram_tensor("out", [NPC, DH], f32, kind="ExternalOutput")

    with tile.TileContext(nc) as tc:
        with tc.tile_pool(name="keep", bufs=1) as keep, \
             tc.tile_pool(name="xgp", bufs=5) as xgp, \
             tc.tile_pool(name="ohp", bufs=5) as ohp, \
             tc.tile_pool(name="oh2p", bufs=8) as oh2p, \
             tc.tile_pool(name="ghslp", bufs=3) as ghslp, \
             tc.tile_pool(name="rot", bufs=2) as rot, \
             tc.tile_pool(name="epi", bufs=2) as epi, \
             tc.tile_pool(name="ps_dn", bufs=4, space="PSUM") as ps_dn, \
             tc.tile_pool(name="ps_ag", bufs=4, space="PSUM") as ps_ag, \
             tc.tile_pool(name="dram", bufs=1, space="DRAM") as dram:

            # ---- preload ----
            # scalar (ACT HWDGE) queue: consts needed by block-0 epilogue.
            cstf = keep.tile([128, 4, DH], f32)
            nc.scalar.dma_start(cstf[:], cstf_in[:])
            cstb = keep.tile([128, 7, DH], bf16)
            nc.scalar.dma_start(cstb[:], cstb_in[:])
            # xT feeds block-0's dense matmul early: SP queue, ahead of xg
            xT = keep.tile([128, NBLK * 128], bf16)
            nc.sync.dma_start(xT[:], xT_in[:])
            # gpsimd queue (idle until the gathers): L2-only inputs.
            idx2 = keep.tile([128, E_pad // 16], i16)
            nc.gpsimd.dma_start(idx2[:], idx2_in[:])
            cnt_t = keep.tile([1, NBLK * 5], mybir.dt.int32)
            nc.gpsimd.dma_start(cnt_t[:], cnt_in[:])

            eps_t = keep.tile([128, 1], f32)
            nc.vector.memset(eps_t[:], LN_EPS)
            ones_t = keep.tile([1, 128], bf16)
            nc.vector.memset(ones_t[:], 1.0)
            ident = keep.tile([128, 128], bf16)
            make_identity(nc, ident[:])
            h_own = keep.tile([128, NBLK * DH], bf16)
            # per-block layer-2 z accumulators [feat_a|feat_b, dst] bf16
            zacc = [keep.tile([128, DH], bf16, name=f"zacc{b}")
                    for b in range(NBLK)]

            hg_self = dram.tile([NPC, DH], hgdt)
            addr_sp = "Shared" if shared else "Local"
            hg_sl = [dram.tile([NC * SLQ[s], DH], hgdt, name=f"hg_sl{s}",
                               addr_space=addr_sp)
                     for s in range(5)]

            g1t = cstf[:, F_G1, :]
            be1t = cstf[:, F_BE1, :]
            g2t = cstf[:, F_G2, :]
            be2t = cstf[:, F_BE2, :]
            b1row = cstb[0:1, B_B1, :]
            bresrow = cstb[0:1, B_BRES, :]
            b2row = cstb[0:1, B_B2, :]
            W1b = cstb[:, B_W1, :]
            W2ab = cstb[:, B_W2A, :]
            W2bb = cstb[:, B_W2B, :]
            Wresb = cstb[:, B_WRES, :]

            # ---- layer-2 gather emission helpers (gpsimd queue) ----
            ghs_t = {}
            gq = [0]

            def emit_slice_gathers(s):
                t = ghslp.tile([128, SCHmax, DH], hgdt, tag="ghs",
                               name=f"ghs{s}")
                ghs_t[s] = t
                nc.vector.memset(t[:], 0)
                # per-core true counts for this slice's 20 gathers;
                # loaded just-in-time so only ~20 regs are live at once
                _, cvals = nc.values_load_multi_w_load_instructions(
                    cnt_t[:, s * NBLK:(s + 1) * NBLK],
                    engines=(mybir.EngineType.Pool,),
                    min_val=0, max_val=int(pad5.max()),
                    skip_runtime_bounds_check=True)
                for b in range(NBLK):
                    kn = int(pad5[b, s]) // 128
                    if kn == 0:
                        continue
                    o16 = int(offs[b] + soff5[b, s]) // 16
                    bo = int(boff[b, s])
                    nc.gpsimd.dma_gather(
                        out_ap=t[:, bo:bo + kn, :], in_ap=hg_sl[s][:],
                        idxs_ap=idx2[:, o16:o16 + int(pad5[b, s]) // 16],
                        num_idxs=int(pad5[b, s]),
                        num_idxs_reg=cvals[b],
                        elem_size=DH, single_packet=False,
                        queue_num=gq[0] % 4)
                    gq[0] += 1

            # ---- layer 1 (no gathers: xg + oht streamed) ----
            P = 4
            xg_t = {}
            oh_t = {}

            def issue_l1(b):
                nchunk = R[b] // 128
                t0 = int(offs[b]) // 128
                xg_t[b] = xgp.tile([128, smax, DIN], bf16, tag="xg",
                                   name=f"xg{b}")
                nc.sync.dma_start(xg_t[b][:, 0:nchunk, :],
                                  xg_in[:, t0:t0 + nchunk, :])
                oh_t[b] = ohp.tile([128, smax, 128], bf16, tag="oh1",
                                   name=f"oh{b}")
                nc.scalar.dma_start(oh_t[b][:, 0:nchunk, :],
                                    oh_in[:, t0:t0 + nchunk, :])

            for b in range(P):
                issue_l1(b)
            for b in range(NBLK):
                if b + P < NBLK:
                    issue_l1(b + P)
                nchunk = R[b] // 128
                xgt = xg_t.pop(b)
                oht = oh_t.pop(b)
                psum_zT = ps_ag.tile([128, 128], f32, tag="agg",
                                     space="PSUM")
                for k in range(nchunk):
                    nc.tensor.matmul(out=psum_zT[:], lhsT=xgt[:, k, :],
                                     rhs=oht[:, k, :], start=(k == 0),
                                     stop=(k == nchunk - 1))
                zts = rot.tile([128, 128], bf16, tag="zts")
                nc.vector.tensor_scalar(out=zts[:], in0=psum_zT[:],
                                        scalar1=0.0, scalar2=None,
                                        op0=mybir.AluOpType.add)
                psum_h1 = ps_dn.tile([128, DH], f32, tag="dense",
                                     space="PSUM")
                nc.tensor.matmul(out=psum_h1[:], lhsT=ones_t[:], rhs=b1row,
                                 start=True, stop=False)
                nc.tensor.matmul(out=psum_h1[:], lhsT=zts[:], rhs=W1b,
                                 start=False, stop=True)
                psum_r = ps_dn.tile([128, DH], f32, tag="dense", space="PSUM")
                nc.tensor.matmul(out=psum_r[:], lhsT=ones_t[:], rhs=bresrow,
                                 start=True, stop=False)
                nc.tensor.matmul(out=psum_r[:],
                                 lhsT=xT[:, b * 128:(b + 1) * 128],
                                 rhs=Wresb, start=False, stop=True)
                delta = epi.tile([128, DH], f32, tag="delta")
                nc.scalar.activation(delta[:], psum_h1[:],
                                     mybir.ActivationFunctionType.Relu)
                u = epi.tile([128, DH], f32, tag="u")
                nc.vector.tensor_tensor(out=u[:], in0=psum_r[:],
                                        in1=delta[:], op=mybir.AluOpType.add)
                hblk = h_own[:, b * DH:(b + 1) * DH]
                _ln(nc, epi, u, g1t, be1t, hblk, eps_t[:])
                hsb = rot.tile([128, DH], hgdt, tag="hsb")
                nc.scalar.activation(hsb[:], hblk,
                                     mybir.ActivationFunctionType.Copy)
                rows = min(128, NPC - b * 128)
                # store on the ACT queue right after the cast executes
                nc.scalar.dma_start(
                    out=hg_self[b * 128:b * 128 + rows, :],
                    in_=hsb[0:rows, :])
                if b in SL_LAST_BLOCK:
                    q = SL_LAST_BLOCK.index(b)
                    lo = SLO[q]
                    nc.gpsimd.collective_compute(
                        "AllGather", mybir.AluOpType.bypass,
                        replica_groups=[list(range(NC))],
                        ins=[hg_self[lo:lo + SLQ[q], :]],
                        outs=[hg_sl[q][:]])
                    if q == 3:
                        emit_slice_gathers(0)
                    elif q == 4:
                        # slots 1/2 first use; s3/s4 are emitted after
                        # the pass that frees their staging slot
                        emit_slice_gathers(1)
                        emit_slice_gathers(2)

            # ---- layer 2: slice-major scatter passes ----
            # per-(block, slice) one-hot loads from the block-major
            # stream (each group is contiguous); small tiles, deep pool,
            # so slot-reuse WARs stay fine-grained.
            kmax5 = int(pad5.max()) // 128
            oh2_t = {}
            PF2 = 6

            def issue_oh2(b, s):
                kn = int(pad5[b, s]) // 128
                if kn == 0:
                    return
                t0 = int(offs[b] + soff5[b, s]) // 128
                t = oh2p.tile([128, kmax5, 128], bf16, tag="oh2",
                              name=f"oh2_{s}_{b}")
                nc.sync.dma_start(t[:, 0:kn, :], oh_in[:, t0:t0 + kn, :])
                oh2_t[(b, s)] = t

            def slice_pass(s):
                ght = ghs_t[s]
                for b in range(PF2):
                    issue_oh2(b, s)
                for b in range(NBLK):
                    if b + PF2 < NBLK:
                        issue_oh2(b + PF2, s)
                    kn = int(pad5[b, s]) // 128
                    if kn == 0:
                        continue
                    bo = int(boff[b, s])
                    oht2 = oh2_t.pop((b, s))
                    psum_a = ps_ag.tile([128, 128], f32, tag="agg",
                                        space="PSUM")
                    psum_b = ps_ag.tile([128, 128], f32, tag="agg",
                                        space="PSUM")
                    for k in range(kn):
                        nc.tensor.matmul(out=psum_a[:],
                                         lhsT=ght[:, bo + k, 0:128],
                                         rhs=oht2[:, k, :],
                                         start=(k == 0), stop=(k == kn - 1))
                        nc.tensor.matmul(out=psum_b[:],
                                         lhsT=ght[:, bo + k, 128:256],
                                         rhs=oht2[:, k, :],
                                         start=(k == 0), stop=(k == kn - 1))
                    za = zacc[b][:, 0:128]
                    zb = zacc[b][:, 128:256]
                    if s == 0:
                        nc.vector.tensor_scalar(out=za, in0=psum_a[:],
                                                scalar1=0.0, scalar2=None,
                                                op0=mybir.AluOpType.add)
                        nc.vector.tensor_scalar(out=zb, in0=psum_b[:],
                                                scalar1=0.0, scalar2=None,
                                                op0=mybir.AluOpType.add)
                    else:
                        nc.vector.tensor_tensor(out=za, in0=psum_a[:],
                                                in1=za,
                                                op=mybir.AluOpType.add)
                        nc.vector.tensor_tensor(out=zb, in0=psum_b[:],
                                                in1=zb,
                                                op=mybir.AluOpType.add)

            # staging slots rotate (s mod 3): gathers for slice s+3 are
            # emitted after pass s so the WAR on the slot is seen
            for s in range(4):
                slice_pass(s)
                if s + 3 < 5:
                    emit_slice_gathers(s + 3)

            # ---- slice-4 session + dense + LN, block-major ----
            s = 4
            ght = ghs_t[s]
            for b in range(3):
                issue_oh2(b, s)
            for b in range(NBLK):
                if b + 3 < NBLK:
                    issue_oh2(b + 3, s)
                kn = int(pad5[b, s]) // 128
                bo = int(boff[b, s])
                oht2 = oh2_t.pop((b, s))
                psum_a = ps_ag.tile([128, 128], f32, tag="agg", space="PSUM")
                psum_b = ps_ag.tile([128, 128], f32, tag="agg", space="PSUM")
                for k in range(kn):
                    nc.tensor.matmul(out=psum_a[:],
                                     lhsT=ght[:, bo + k, 0:128],
                                     rhs=oht2[:, k, :],
                                     start=(k == 0), stop=(k == kn - 1))
                    nc.tensor.matmul(out=psum_b[:],
                                     lhsT=ght[:, bo + k, 128:256],
                                     rhs=oht2[:, k, :],
                                     start=(k == 0), stop=(k == kn - 1))
                za = zacc[b][:, 0:128]
                zb = zacc[b][:, 128:256]
                nc.vector.tensor_tensor(out=za, in0=psum_a[:], in1=za,
                                        op=mybir.AluOpType.add)
                nc.vector.tensor_tensor(out=zb, in0=psum_b[:], in1=zb,
                                        op=mybir.AluOpType.add)
                psum_d2 = ps_dn.tile([128, DH], f32, tag="dense",
                                     space="PSUM")
                nc.tensor.matmul(out=psum_d2[:], lhsT=ones_t[:], rhs=b2row,
                                 start=True, stop=False)
                nc.tensor.matmul(out=psum_d2[:], lhsT=za, rhs=W2ab,
                                 start=False, stop=False)
                nc.tensor.matmul(out=psum_d2[:], lhsT=zb, rhs=W2bb,
                                 start=False, stop=False)
                nc.tensor.matmul(out=psum_d2[:], lhsT=ident[:],
                                 rhs=h_own[:, b * DH:(b + 1) * DH],
                                 start=False, stop=True)
                outb = epi.tile([128, DH], f32, tag="outb")
                _ln(nc, epi, psum_d2, g2t, be2t, outb, eps_t[:])
                rows = min(128, NPC - b * 128)
                nc.sync.dma_start(out=out_t[b * 128:b * 128 + rows, :],
                                  in_=outb[0:rows, :])
    nc.compile()
    return nc


def _host_prep(edge_index, edge_weight):
    """Edge preprocessing: self-loops folded in, per-(core, block) edge
    lists sorted by the AG slice of the source node (5 groups), each
    group padded to a chunk multiple (pad slots get src=-1, norm=0)."""
    src = np.asarray(edge_index[0], np.int64)
    dst = np.asarray(edge_index[1], np.int64)
    w = np.asarray(edge_weight, np.float32)
    deg = np.ones(N, np.float32)  # self-loop weight 1 included
    np.add.at(deg, dst, w)
    dinv = np.where(deg > 0, 1.0 / np.sqrt(deg), 0.0).astype(np.float32)
    loop = np.arange(N, dtype=np.int64)
    src_a = np.concatenate([src, loop])
    dst_a = np.concatenate([dst, loop])
    norm_a = np.concatenate([(dinv[src] * w * dinv[dst]).astype(np.float32),
                             (dinv * dinv).astype(np.float32)])

    sslice = np.minimum((src_a % NPC) // 512, 4)
    # sort by (dst block, src slice); stable so layout is deterministic
    bkey = (dst_a // NPC) * NBLK + (dst_a % NPC) // 128
    order = np.lexsort((sslice, bkey))
    src_s, dst_s, norm_s, ss_s = (src_a[order], dst_a[order],
                                  norm_a[order], sslice[order])
    bkey_s = bkey[order]

    core_id = dst_s // NPC
    brel = (dst_s % NPC) // 128
    cnt5 = np.zeros((NC, NBLK, 5), np.int64)
    np.add.at(cnt5, (core_id, brel, ss_s), 1)
    # pad each (block, slice) group to a chunk multiple, max over cores
    pad5 = (np.ceil(cnt5.max(axis=0) / 128) * 128).astype(np.int64)
    R = tuple(int(v) for v in pad5.sum(axis=1))
    offs = np.concatenate([[0], np.cumsum(R)]).astype(np.int64)
    E_pad = int(offs[-1])
    soff5 = np.zeros((NBLK, 5), np.int64)
    soff5[:, 1:] = np.cumsum(pad5, axis=1)[:, :-1]

    src_pad = np.full((NC, E_pad), -1, np.int64)
    dstrel_pad = np.zeros((NC, E_pad), np.int64)
    wn_pad = np.zeros((NC, E_pad), np.float32)
    real = np.zeros((NC, E_pad), bool)
    blk_lo = np.searchsorted(bkey_s, np.arange(NC * NBLK), 'left')
    for c in range(NC):
        for b in range(NBLK):
            i = c * NBLK + b
            lo = blk_lo[i]
            base = c * NPC + b * 128
            for s in range(5):
                n = int(cnt5[c, b, s])
                o = int(offs[b] + soff5[b, s])
                src_pad[c, o:o + n] = src_s[lo:lo + n]
                dstrel_pad[c, o:o + n] = dst_s[lo:lo + n] - base
                wn_pad[c, o:o + n] = norm_s[lo:lo + n]
                real[c, o:o + n] = True
                lo += n
    return pad5, soff5, cnt5, src_pad, dstrel_pad, wn_pad, real


def kernel(x, edge_index, edge_weight, W1, b1, W2, b2, Wres, bres,
           gamma1, beta1, gamma2, beta2):
    global LAST_EXEC_TIME_NS
    x = np.ascontiguousarray(np.asarray(x, np.float32))
    W1 = np.asarray(W1, np.float32)
    W2 = np.asarray(W2, np.float32)
    Wres = np.asarray(Wres, np.float32)

    hgdt_name = os.environ.get("GCN_HG_DT", "fp8")
    shared = os.environ.get("GCN_SHARED", "0") == "1"

    (pad5, soff5, cnt5, src_pad, dstrel_pad, wn_pad,
     real) = _host_prep(edge_index, edge_weight)
    R = tuple(int(v) for v in pad5.sum(axis=1))
    offs = np.concatenate([[0], np.cumsum(R)]).astype(np.int64)
    E_pad = int(offs[-1])
    CH = E_pad // 128

    cstf = np.zeros((128, 4, DH), np.float32)
    cstf[:, F_G1, :] = np.asarray(gamma1, np.float32)[None, :]
    cstf[:, F_BE1, :] = np.asarray(beta1, np.float32)[None, :]
    cstf[:, F_G2, :] = np.asarray(gamma2, np.float32)[None, :]
    cstf[:, F_BE2, :] = np.asarray(beta2, np.float32)[None, :]
    cstb = np.zeros((128, 7, DH), np.float32)
    cstb[:, B_W1, :] = W1
    cstb[:, B_W2A, :] = W2[:128, :]
    cstb[:, B_W2B, :] = W2[128:, :]
    cstb[:, B_WRES, :] = Wres
    cstb[:, B_B1, :] = np.asarray(b1, np.float32)[None, :]
    cstb[:, B_BRES, :] = np.asarray(bres, np.float32)[None, :]
    cstb[:, B_B2, :] = np.asarray(b2, np.float32)[None, :]
    cstb = cstb.astype(ml_dtypes.bfloat16)

    xbf = x.astype(ml_dtypes.bfloat16)

    # row of each global node inside its slice tensor hg_sl[s]
    n_all = np.arange(N, dtype=np.int64)
    c_all = n_all // NPC
    l_all = n_all % NPC
    q_all = np.minimum(l_all // 512, 4)
    slq = np.asarray(SLQ, np.int64)
    slo = np.asarray(SLO, np.int64)
    rowin = c_all * slq[q_all] + (l_all - slo[q_all])

    in_maps = []
    for c in range(NC):
        sp = src_pad[c]
        valid = sp >= 0
        idx2_flat = np.where(valid, rowin[np.where(valid, sp, 0)],
                             -1).astype(np.int16)
        idx2_w = np.tile(idx2_flat.reshape(E_pad // 16, 16).T, (8, 1)).copy()
        # per-edge x stream [128, CH, 128] bf16 (pads are zero rows)
        xg = np.zeros((E_pad, DIN), ml_dtypes.bfloat16)
        xg[valid] = xbf[sp[valid]]
        xg = np.ascontiguousarray(
            xg.reshape(CH, 128, DIN).transpose(1, 0, 2))
        # one-hot scatter tiles [128, CH, 128]
        oh = np.zeros((E_pad, 128), np.float32)
        rr = real[c]
        oh[np.nonzero(rr)[0], dstrel_pad[c][rr]] = wn_pad[c][rr]
        oh = np.ascontiguousarray(oh.reshape(CH, 128, 128).transpose(1, 0, 2))
        ohb = oh.astype(ml_dtypes.bfloat16)
        xT = np.zeros((128, NBLK * 128), np.float32)
        xT[:, :NPC] = x[c * NPC:(c + 1) * NPC].T
        cnt5c = np.ascontiguousarray(
            cnt5[c].T.reshape(1, NBLK * 5)).astype(np.int32)
        in_maps.append({
            "xg": xg,
            "ohb": ohb,
            "idx2": idx2_w,
            "xT": xT.astype(ml_dtypes.bfloat16),
            "cstf": cstf,
            "cstb": cstb,
            "cnt5": cnt5c,
        })

    key = (tuple(map(tuple, pad5)), hgdt_name, shared)
    nc = _prog_cache.get(key)
    if nc is None:
        nc = _build_program(pad5, hgdt_name, shared)
        _prog_cache[key] = nc

    trace = bool(os.environ.get("BASS_KERNEL_TRACE"))
    res = run_bass_kernel_spmd(nc, in_maps, list(range(NC)), trace=trace)
    if trace:
        LAST_EXEC_TIME_NS = res.exec_time_ns
    out = np.concatenate([res.results[c]["out"] for c in range(NC)], axis=0)
    return np.ascontiguousarray(out.astype(np.float32))
